# revision 10
# baseline (speedup 1.0000x reference)
"""GPT (4-layer, B=4 T=1024 C=1024 NH=8) Trainium2 Bass kernel, v2.

Sharding: 8 cores = 4 batches (DP) x 2 interleaved token-tile sets (SP).
Even cores own token tiles {1,2,4,7} of their batch, odd cores {0,3,5,6}
(balanced causal work: each set needs 18 of 36 causal 128x128 tiles).

Per layer: LN1 -> K,V projections per head-group (4 heads) -> AllGather
kv (fp8) per group (2 smaller collectives overlap with Q projection and
the first group's attention) -> attention with variable-width causal
slabs -> out-projection -> LN2 -> MLP (bf16) -> residual.

Precision: residual fp32 (+bf16 shadow for LN sums), QKV/proj matmuls
fp8e4 DoubleRow (2 k-tiles per MM), attention fp8 (scores/AV normal
mode), MLP and head bf16 (fp8 there fails the 2e-2 gate: no softmax
averaging to absorb activation-quantization noise).

Softmax: exp on scalar engine with fused per-partition column-disable
bias; causal masking only on the first 128-col slab of each position via
per-core mask data (triangle/ones/zeros); denominators via
reciprocal_approx_fast + PE-broadcast (no DRAM round-trips).
"""

import os
import sys

import numpy as np

for _p in ("/opt/trn_rl_repo",):
    if _p not in sys.path and os.path.isdir(_p):
        sys.path.insert(0, _p)

import ml_dtypes  # noqa: E402

BF16 = ml_dtypes.bfloat16
FP8 = ml_dtypes.float8_e4m3

# model dims
B, T, C, L = 4, 1024, 1024, 4
NH, HD = 8, 128
HID = 4 * C
V1 = 101  # V + 1
TD = 64  # TOTAL_DIM
NUM_NODE, F_DIM = 15, 4
D_BIAS = NUM_NODE * F_DIM  # 60
EPS = 1e-5
NCORES = 8
TOK = T // 2  # 512 tokens per core
KSUB = C // 128  # 8
HSUB = HID // 128  # 32
SQHD = float(np.sqrt(HD))  # 11.31

WS = 16.0  # fp8 weight prescale (folded back after matmul)

# token-tile assignment: even cores set A, odd cores set B
TILES_A = [1, 2, 4, 7]
TILES_B = [0, 3, 5, 6]
# gathered k-tile order = [A tiles; B tiles]
GATH = TILES_A + TILES_B
# per-position slab start (cols S..512 computed at position p)
SLAB = [0, 128, 256, 384, 0, 128, 256, 384]

_CACHED = {}


def _build_program(fp8=True):
    import concourse.bacc as bacc
    import concourse.bass as bass  # noqa: F401
    import concourse.mybir as mybir
    import concourse.tile as tile

    dt = mybir.dt
    AF = mybir.ActivationFunctionType
    OP = mybir.AluOpType
    DR = mybir.MatmulPerfMode.DoubleRow if fp8 else None
    f8 = dt.float8e4 if fp8 else dt.bfloat16

    nc = bacc.Bacc(trn_type="TRN2", num_devices=NCORES)

    # ---- I/O ----
    h0T_d = nc.dram_tensor("h0T", (128, KSUB, TOK), dt.bfloat16, kind="ExternalInput")
    dm_d = nc.dram_tensor("dm", (128, 8, 128), dt.bfloat16, kind="ExternalInput")
    pbT_d = nc.dram_tensor("pbT", (64, NH, 64), dt.float32, kind="ExternalInput")
    cb_d = nc.dram_tensor("cb", (128, 1), dt.float32, kind="ExternalInput")
    wq_d = nc.dram_tensor("wq", (L, 128, KSUB, C), f8, kind="ExternalInput")
    wk_d = nc.dram_tensor("wk", (L, 128, KSUB, C), f8, kind="ExternalInput")
    wv_d = nc.dram_tensor("wv", (L, 128, KSUB, C), f8, kind="ExternalInput")
    wp_d = nc.dram_tensor("wp", (L, 128, KSUB, C), f8, kind="ExternalInput")
    w1_d = nc.dram_tensor("w1", (L, 8, 128, KSUB, 512), dt.bfloat16, kind="ExternalInput")
    w2_d = nc.dram_tensor("w2", (L, 4, HSUB, 128, 256), dt.bfloat16, kind="ExternalInput")
    hwT_d = nc.dram_tensor("hwT", (TD, 128, KSUB, 128), dt.bfloat16, kind="ExternalInput")
    out_d = nc.dram_tensor("logits", (TD, TOK // TD, V1), dt.float32, kind="ExternalOutput")

    RG = [[0, 1], [2, 3], [4, 5], [6, 7]]
    KVQ = 4 * HD * TOK  # elements per k (or v) group shard

    with tile.TileContext(nc) as tc:
        consts = tc.alloc_tile_pool(name="consts", bufs=1)
        hpool = tc.alloc_tile_pool(name="hpool", bufs=1)
        wpool = tc.alloc_tile_pool(name="wpool", bufs=2)
        w1pool = tc.alloc_tile_pool(name="w1pool", bufs=2)
        w2pool = tc.alloc_tile_pool(name="w2pool", bufs=4)
        apool = tc.alloc_tile_pool(name="apool", bufs=2)
        dpool = tc.alloc_tile_pool(name="dpool", bufs=2, space="DRAM")
        # PSUM: acc 4 banks (persistent accumulators), s 2 (transient mm),
        # v 2 (reductions/broadcasts) = 8 banks
        ps_acc = tc.alloc_tile_pool(name="ps_acc", bufs=2, space="PSUM")
        ps_s = tc.alloc_tile_pool(name="ps_s", bufs=4, space="PSUM")
        ps_v = tc.alloc_tile_pool(name="ps_v", bufs=2, space="PSUM")

        # ---- constants ----
        dm_sb = consts.tile([128, 8, 128], dt.bfloat16, name="dm_sb")
        nc.sync.dma_start(dm_sb, dm_d[:])
        pb_sb = consts.tile([64, NH, 64], dt.float32, name="pb_sb")
        nc.sync.dma_start(pb_sb, pbT_d[:])
        cb_sb = consts.tile([128, 1], dt.float32, name="cb_sb")
        nc.sync.dma_start(cb_sb, cb_d[:])
        ones8 = consts.tile([128, 1], f8, name="ones8")
        nc.vector.memset(ones8, 1.0)
        ones16 = consts.tile([128, 1], dt.bfloat16, name="ones16")
        nc.vector.memset(ones16, 1.0)
        onesrow = consts.tile([1, 128], dt.bfloat16, name="onesrow")
        nc.vector.memset(onesrow, 1.0)

        # ---- residual stream: bf16 ----
        h16 = hpool.tile([128, KSUB, TOK], dt.bfloat16, name="h16")
        nc.sync.dma_start(h16, h0T_d[:])

        def layernorm(dst, dst_dt_scale=None):
            """LN over C of h16 -> dst.  Stats bf16, sums via PE."""
            sum_ps = ps_v.tile([1, TOK], dt.float32, tag="v", name="sum_ps")
            for ct in range(KSUB):
                nc.tensor.matmul(
                    sum_ps, ones16, h16[:, ct, :],
                    start=(ct == 0), stop=(ct == KSUB - 1),
                )
            ssq_ps = ps_v.tile([1, TOK], dt.float32, tag="v", name="ssq_ps")
            for ct in range(KSUB):
                sq16 = apool.tile([128, TOK], dt.bfloat16, tag="sq", name="sq16")
                nc.vector.tensor_mul(sq16, h16[:, ct, :], h16[:, ct, :])
                nc.tensor.matmul(
                    ssq_ps, ones16, sq16,
                    start=(ct == 0), stop=(ct == KSUB - 1),
                )
            stat = apool.tile([1, 3 * TOK], dt.float32, tag="stat", name="stat")
            mean = stat[:, 0:TOK]
            var = stat[:, TOK : 2 * TOK]
            rvar = stat[:, 2 * TOK : 3 * TOK]
            nc.vector.tensor_scalar_mul(mean, sum_ps, 1.0 / C)
            msq = apool.tile([1, TOK], dt.float32, tag="msq", name="msq")
            nc.vector.tensor_mul(msq, mean, mean)
            # var = ssq/C - mean^2 + eps
            nc.vector.scalar_tensor_tensor(
                var, ssq_ps, 1.0 / C, msq, OP.mult, OP.subtract
            )
            nc.vector.tensor_scalar_add(var, var, EPS)
            nc.vector.reciprocal_approx_fast(rvar, var)
            st16 = apool.tile([1, 2 * TOK], dt.bfloat16, tag="st16", name="st16")
            rstd16 = st16[:, 0:TOK]
            mr16 = st16[:, TOK : 2 * TOK]
            nc.scalar.activation(rstd16, rvar, AF.Sqrt)  # 1/sqrt(var)
            nc.vector.tensor_mul(mr16, mean, rstd16)
            # broadcast via PE (K=1 matmul)
            rstd_b = ps_v.tile([128, TOK], dt.float32, tag="v", name="rstd_b")
            nc.tensor.matmul(rstd_b, onesrow, rstd16, start=True, stop=True)
            mr_b = ps_v.tile([128, TOK], dt.float32, tag="v", name="mr_b")
            nc.tensor.matmul(mr_b, onesrow, mr16, start=True, stop=True)
            for ct in range(KSUB):
                tmp = apool.tile([128, TOK], dt.float32, tag="lntmp", name="lntmp")
                nc.vector.tensor_mul(tmp, h16[:, ct, :], rstd_b)
                nc.vector.tensor_sub(dst[:, ct, :], tmp, mr_b)

        def resid_add(co, pd, scale):
            """h16[:, co] += pd*scale (in place)."""
            if scale != 1.0:
                nc.vector.scalar_tensor_tensor(
                    h16[:, co, :], pd, scale, h16[:, co, :], OP.mult, OP.add
                )
            else:
                nc.vector.tensor_add(h16[:, co, :], h16[:, co, :], pd)

        IWS = (1.0 / WS) if fp8 else 1.0

        for layer in range(L):
            # ---------- LN1 ----------
            aT = apool.tile([128, KSUB, TOK], f8, tag="aT", name="aT")
            layernorm(aT)

            # ---------- K, V projections per group + AllGather ----------
            wk_sb = wpool.tile([128, KSUB, C], f8, tag="wmat", name="wk_sb")
            nc.sync.dma_start(wk_sb, wk_d[layer])
            wv_sb = wpool.tile([128, KSUB, C], f8, tag="wmat", name="wv_sb")
            nc.sync.dma_start(wv_sb, wv_d[layer])

            kv_ga = []
            for grp in range(2):
                kv_in = dpool.tile([2, KVQ], f8, tag="kvin", name="kv_in")
                kv_g = dpool.tile([2, 2, KVQ], f8, tag="kvga", name="kv_ga")
                kin = kv_in[0].rearrange("(c t) -> c t", t=TOK)  # (512, TOK) head-major
                vin = kv_in[1].rearrange("(t c) -> t c", c=4 * HD)  # (TOK, 512) tok-major
                for hl in range(4):
                    hh = grp * 4 + hl
                    pk = ps_s.tile([128, TOK], dt.float32, tag="s", name="pk")
                    if fp8:
                        for gp in range(4):
                            nc.tensor.matmul(
                                pk,
                                wk_sb[:, 2 * gp : 2 * gp + 2, hh * HD : (hh + 1) * HD],
                                aT[:, 2 * gp : 2 * gp + 2, :],
                                start=(gp == 0), stop=(gp == 3), perf_mode=DR,
                            )
                    else:
                        for ct in range(KSUB):
                            nc.tensor.matmul(
                                pk, wk_sb[:, ct, hh * HD : (hh + 1) * HD],
                                aT[:, ct, :],
                                start=(ct == 0), stop=(ct == KSUB - 1),
                            )
                    k8 = apool.tile([128, TOK], f8, tag="kv8", name="k8")
                    nc.scalar.activation(k8, pk, AF.Copy, scale=IWS)
                    nc.sync.dma_start(kin[hl * HD : (hl + 1) * HD, :], k8)
                for tsub in range(4):
                    pv = ps_s.tile([128, 512], dt.float32, tag="s", name="pv")
                    if fp8:
                        for gp in range(4):
                            nc.tensor.matmul(
                                pv,
                                aT[:, 2 * gp : 2 * gp + 2, tsub * 128 : (tsub + 1) * 128],
                                wv_sb[:, 2 * gp : 2 * gp + 2, grp * 512 : (grp + 1) * 512],
                                start=(gp == 0), stop=(gp == 3), perf_mode=DR,
                            )
                    else:
                        for ct in range(KSUB):
                            nc.tensor.matmul(
                                pv,
                                aT[:, ct, tsub * 128 : (tsub + 1) * 128],
                                wv_sb[:, ct, grp * 512 : (grp + 1) * 512],
                                start=(ct == 0), stop=(ct == KSUB - 1),
                            )
                    v8 = apool.tile([128, 512], f8, tag="kv8", name="v8")
                    nc.scalar.activation(v8, pv, AF.Copy, scale=IWS)
                    nc.sync.dma_start(vin[tsub * 128 : (tsub + 1) * 128, :], v8)
                nc.gpsimd.collective_compute(
                    "AllGather",
                    OP.bypass,
                    replica_groups=RG,
                    ins=[kv_in.opt()],
                    outs=[kv_g.opt()],
                )
                kv_ga.append(kv_g)

            # ---------- Q projection (overlaps AllGathers) ----------
            wq_sb = wpool.tile([128, KSUB, C], f8, tag="wmat", name="wq_sb")
            nc.sync.dma_start(wq_sb, wq_d[layer])
            qT = apool.tile([128, NH, TOK], f8, tag="qT", name="qT", bufs=1)
            for hh in range(NH):
                pq = ps_s.tile([128, TOK], dt.float32, tag="s", name="pq")
                if fp8:
                    for gp in range(4):
                        nc.tensor.matmul(
                            pq,
                            wq_sb[:, 2 * gp : 2 * gp + 2, hh * HD : (hh + 1) * HD],
                            aT[:, 2 * gp : 2 * gp + 2, :],
                            start=(gp == 0), stop=(gp == 3), perf_mode=DR,
                        )
                else:
                    for ct in range(KSUB):
                        nc.tensor.matmul(
                            pq, wq_sb[:, ct, hh * HD : (hh + 1) * HD], aT[:, ct, :],
                            start=(ct == 0), stop=(ct == KSUB - 1),
                        )
                nc.scalar.activation(qT[:, hh, :], pq, AF.Copy, scale=IWS)

            # ---------- attention ----------
            yT = apool.tile([128, NH, TOK], f8, tag="yT", name="yT", bufs=1)
            for grp in range(2):
                kv_g = kv_ga[grp]
                kg_all = apool.tile([128, 2, 4, TOK], f8, tag="kg", name="kg_all")
                vg_all = apool.tile([128, 8, 512], f8, tag="vg", name="vg_all")
                for r in range(2):
                    kga = kv_g[r, 0].rearrange("(hl hd t) -> hd hl t", hd=HD, t=TOK)
                    vga = kv_g[r, 1].rearrange("(ts t c) -> t ts c", t=128, c=4 * HD)
                    nc.sync.dma_start(kg_all[:, r, :, :], kga)
                    nc.sync.dma_start(vg_all[:, r * 4 : (r + 1) * 4, :], vga)
                for hl in range(4):
                    hh = grp * 4 + hl

                    e_sb = apool.tile([128, 8, TOK], f8, tag="e_sb", name="e_sb", bufs=3)
                    den_ps = ps_v.tile([1, TOK], dt.float32, tag="v", name="den_ps")
                    py = ps_acc.tile([128, TOK], dt.float32, tag="acc", name="py")

                    def den_av(p):
                        # den/AV accumulation, deferred 3 positions behind the
                        # scores so the PE never blocks on exp (FIFO queues).
                        # AV: odd p pairs with p-1 in DoubleRow over the
                        # narrower suffix; the 128-col head of the even
                        # position is done in normal mode.
                        S = SLAB[p]
                        nc.tensor.matmul(
                            den_ps[:, S:TOK], ones8, e_sb[:, p, S:TOK],
                            start=(p == 0), stop=(p == 7),
                        )
                        if fp8:
                            if p % 2 == 0:
                                nc.tensor.matmul(
                                    py[:, S : S + 128],
                                    vg_all[:, p, hl * HD : (hl + 1) * HD],
                                    e_sb[:, p, S : S + 128],
                                    start=(p == 0), stop=False,
                                )
                            else:
                                nc.tensor.matmul(
                                    py[:, S:TOK],
                                    vg_all[:, p - 1 : p + 1, hl * HD : (hl + 1) * HD],
                                    e_sb[:, p - 1 : p + 1, S:TOK],
                                    start=False, stop=(p == 7), perf_mode=DR,
                                )
                        else:
                            nc.tensor.matmul(
                                py[:, S:TOK],
                                vg_all[:, p, hl * HD : (hl + 1) * HD],
                                e_sb[:, p, S:TOK],
                                start=(p == 0), stop=(p == 7),
                            )

                    for p in range(8):
                        S = SLAB[p]
                        s_ps = ps_s.tile([128, TOK], dt.float32, tag="s", name="s_ps")
                        nc.tensor.matmul(
                            s_ps[:, S:TOK],
                            kg_all[:, p // 4, hl, (p % 4) * 128 : (p % 4 + 1) * 128],
                            qT[:, hh, S:TOK],
                            start=True, stop=True,
                        )
                        if p == 4:
                            nc.vector.tensor_add(
                                s_ps[0:64, 0:64], s_ps[0:64, 0:64], pb_sb[:, hh, :]
                            )
                        nc.scalar.activation(
                            e_sb[:, p, S:TOK], s_ps[:, S:TOK], AF.Exp,
                            bias=cb_sb, scale=1.0 / SQHD,
                        )
                        nc.vector.tensor_mul(
                            e_sb[:, p, S : S + 128],
                            e_sb[:, p, S : S + 128],
                            dm_sb[:, p, :],
                        )
                        if p >= 3:
                            den_av(p - 3)
                    for p in range(5, 8):
                        den_av(p)
                    den16 = apool.tile([1, TOK], dt.bfloat16, tag="den16", name="den16")
                    nc.vector.tensor_copy(den16, den_ps)
                    den_b = ps_v.tile([128, TOK], dt.float32, tag="v", name="den_b")
                    nc.tensor.matmul(den_b, onesrow, den16, start=True, stop=True)
                    recB = apool.tile([128, TOK], dt.float32, tag="recB", name="recB")
                    nc.vector.reciprocal_approx_fast(recB, den_b)
                    nc.vector.tensor_mul(yT[:, hh, :], py, recB)

            # ---------- proj + residual ----------
            wp_sb = wpool.tile([128, KSUB, C], f8, tag="wmat", name="wp_sb")
            nc.sync.dma_start(wp_sb, wp_d[layer])
            for co in range(KSUB):
                pp = ps_s.tile([128, TOK], dt.float32, tag="s", name="pp")
                if fp8:
                    for gp in range(4):
                        nc.tensor.matmul(
                            pp,
                            wp_sb[:, 2 * gp : 2 * gp + 2, co * 128 : (co + 1) * 128],
                            yT[:, 2 * gp : 2 * gp + 2, :],
                            start=(gp == 0), stop=(gp == 3), perf_mode=DR,
                        )
                else:
                    for ct in range(KSUB):
                        nc.tensor.matmul(
                            pp, wp_sb[:, ct, co * 128 : (co + 1) * 128], yT[:, ct, :],
                            start=(ct == 0), stop=(ct == KSUB - 1),
                        )
                resid_add(co, pp, IWS)

            # ---------- LN2 ----------
            a2T = apool.tile([128, KSUB, TOK], dt.bfloat16, tag="aT", name="a2T")
            layernorm(a2T)

            # ---------- MLP (bf16) ----------
            g_sb = apool.tile([128, HSUB, TOK], dt.bfloat16, tag="g_sb", name="g_sb",
                              bufs=1)
            for hblk in range(8):
                w1_sb = w1pool.tile([128, KSUB, 512], dt.bfloat16, tag="w1b",
                                    name="w1_sb")
                nc.sync.dma_start(w1_sb, w1_d[layer, hblk])
                for hc in range(4):
                    pu = ps_s.tile([128, TOK], dt.float32, tag="s", name="pu")
                    for ct in range(KSUB):
                        nc.tensor.matmul(
                            pu,
                            w1_sb[:, ct, hc * 128 : (hc + 1) * 128],
                            a2T[:, ct, :],
                            start=(ct == 0), stop=(ct == KSUB - 1),
                        )
                    nc.scalar.activation(g_sb[:, hblk * 4 + hc, :], pu, AF.Gelu)

            for grp2 in range(4):
                pd = [
                    ps_acc.tile([128, TOK], dt.float32, tag="acc", name=f"pd{i}")
                    for i in range(2)
                ]
                for ksub in range(HSUB):
                    w2_sb = w2pool.tile([128, 256], dt.bfloat16, tag="w2t",
                                        name="w2_sb")
                    nc.sync.dma_start(w2_sb, w2_d[layer, grp2, ksub])
                    for i in range(2):
                        nc.tensor.matmul(
                            pd[i],
                            w2_sb[:, i * 128 : (i + 1) * 128],
                            g_sb[:, ksub, :],
                            start=(ksub == 0), stop=(ksub == HSUB - 1),
                        )
                for i in range(2):
                    resid_add(grp2 * 2 + i, pd[i], 1.0)

        # ---------- final LN + head (bf16) ----------
        hfT = apool.tile([128, KSUB, TOK], dt.bfloat16, tag="aT", name="hfT")
        layernorm(hfT)
        hfT_r = hfT.rearrange("p k (b e) -> p k e b", e=TD)  # b: 8 blocks of 64
        NB = TOK // TD  # 8 tokens per class
        for eg in range(TD // 4):  # 4 e-classes concurrently via PE col groups
            hw4 = [
                w1pool.tile([128, KSUB, 128], dt.bfloat16, tag=f"hw{j}",
                            name=f"hw4_{j}", bufs=2)
                for j in range(4)
            ]
            for j in range(4):
                nc.sync.dma_start(hw4[j], hwT_d[4 * eg + j])
            po4 = ps_s.tile([128, V1], dt.float32, tag="s", name="po4")
            for ct in range(KSUB):
                for j in range(4):
                    nc.tensor.matmul(
                        po4[32 * j : 32 * j + NB, :],
                        hfT_r[:, ct, 4 * eg + j, :],
                        hw4[j][:, ct, 0:V1],
                        start=(ct == 0), stop=(ct == KSUB - 1),
                        tile_position=(0, 32 * j),
                    )
            o_sb = apool.tile([128, V1], dt.float32, tag="o_sb", name="o_sb")
            nc.vector.tensor_copy(o_sb, po4)
            for j in range(4):
                nc.sync.dma_start(out_d[4 * eg + j], o_sb[32 * j : 32 * j + NB, :])

        for p in (ps_v, ps_s, ps_acc, dpool, apool, w2pool, w1pool, wpool, hpool,
                  consts):
            p.release()

    nc.compile()
    return nc


def _tok_idx(half):
    tiles = TILES_A if half == 0 else TILES_B
    return np.concatenate([np.arange(t * 128, (t + 1) * 128) for t in tiles])


def _host_inputs(x, attn_bias, pos_emb, Wq, Wk, Wv, Wp, w1, w2, head_w, fp8=True):
    f8c = (lambda a: np.clip(a, -240, 240).astype(FP8)) if fp8 else (
        lambda a: a.astype(BF16))

    def packw(W):
        # (L, C, N) -> (L, 128, KSUB, N): partition-major contiguous
        W = np.asarray(W, np.float32)
        n = W.shape[2]
        return np.ascontiguousarray(
            W.reshape(L, KSUB, 128, n).transpose(0, 2, 1, 3))

    sc = WS if fp8 else 1.0
    wq = f8c(packw(Wq) * sc)
    wk = f8c(packw(Wk) * sc)
    wv = f8c(packw(Wv) * sc)
    wp = f8c(packw(Wp) * sc)
    w1b = np.ascontiguousarray(
        np.asarray(w1, np.float32).reshape(L, KSUB, 128, 8, 512)
        .transpose(0, 3, 2, 1, 4)).astype(BF16)
    w2b = np.ascontiguousarray(
        np.asarray(w2, np.float32).reshape(L, HSUB, 128, 4, 256)
        .transpose(0, 3, 1, 2, 4)).astype(BF16)
    hwT = np.zeros((TD, C, 128), np.float32)
    hwT[:, :, :V1] = np.asarray(head_w, np.float32).transpose(0, 2, 1)
    hwT = np.ascontiguousarray(
        hwT.reshape(TD, KSUB, 128, 128).transpose(0, 2, 1, 3)).astype(BF16)

    # graph bias, transposed (kv, head, q), padded 60->64, prescaled by
    # sqrt(HD) (the kernel applies exp(s/sqrt(HD) + colbias))
    bias = np.repeat(np.repeat(np.asarray(attn_bias, np.float32), F_DIM, 1), F_DIM, 2)
    pbT = np.zeros((64, NH, 64), np.float32)
    pbT[:D_BIAS, :, :D_BIAS] = bias.transpose(2, 0, 1) * SQHD  # [j, h, i]
    pbT_zero = np.zeros_like(pbT)

    # column-disable bias: kv rows with global_j % 64 == 63 get -50
    cb = np.zeros((128, 1), np.float32)
    cb[63, 0] = -50.0
    cb[127, 0] = -50.0

    # per-position first-slab masks [128(k), 8(pos), 128(q)]
    tri = np.tril(np.ones((128, 128), np.float32)).T  # tri[k, q] = k <= q
    onesm = np.ones((128, 128), np.float32)
    zerom = np.zeros((128, 128), np.float32)
    dm_A = np.stack([tri, tri, tri, tri, onesm, zerom, zerom, onesm], 1)
    dm_B = np.stack([zerom, onesm, onesm, zerom, tri, tri, tri, tri], 1)
    dm_A = np.ascontiguousarray(dm_A).astype(BF16)
    dm_B = np.ascontiguousarray(dm_B).astype(BF16)

    h0 = np.asarray(x, np.float32) + np.asarray(pos_emb, np.float32)  # (B, T, C)

    in_maps = []
    for core in range(NCORES):
        b, half = core // 2, core % 2
        idx = _tok_idx(half)
        h0T = np.ascontiguousarray(
            h0[b, idx].T.reshape(KSUB, 128, TOK).transpose(1, 0, 2)
        ).astype(BF16)  # (128, KSUB, TOK)
        in_maps.append(
            {
                "h0T": h0T,
                "dm": dm_A if half == 0 else dm_B,
                "pbT": pbT_zero if half == 0 else pbT,
                "cb": cb,
                "wq": wq, "wk": wk, "wv": wv, "wp": wp,
                "w1": w1b, "w2": w2b, "hwT": hwT,
            }
        )
    return in_maps


def kernel(**inputs):
    from concourse.bass_utils import run_bass_kernel_spmd

    fp8 = bool(int(os.environ.get("KERNEL_FP8", "1")))
    in_maps = _host_inputs(
        inputs["x"], inputs["attn_bias"], inputs["pos_emb"],
        inputs["Wq"], inputs["Wk"], inputs["Wv"], inputs["Wp"],
        inputs["w1"], inputs["w2"], inputs["head_w"], fp8=fp8,
    )
    if "nc" not in _CACHED:
        _CACHED["nc"] = _build_program(fp8=fp8)
    res = run_bass_kernel_spmd(
        _CACHED["nc"], in_maps, core_ids=list(range(NCORES)),
        trace=bool(int(os.environ.get("KERNEL_TRACE", "0"))),
    )
    out = np.zeros((B, T, V1), np.float32)
    for core in range(NCORES):
        b, half = core // 2, core % 2
        lg = res.results[core]["logits"]  # (TD, 8, V1): token = b*64 + e
        lg = lg.transpose(1, 0, 2).reshape(TOK, V1)
        out[b, _tok_idx(half)] = lg
    _CACHED["last_result"] = res
    return out


# revision 11
# speedup vs baseline: 1.0073x; 1.0073x over previous
"""GPT (4-layer, B=4 T=1024 C=1024 NH=8) Trainium2 Bass kernel, v2.

Sharding: 8 cores = 4 batches (DP) x 2 interleaved token-tile sets (SP).
Even cores own token tiles {1,2,4,7} of their batch, odd cores {0,3,5,6}
(balanced causal work: each set needs 18 of 36 causal 128x128 tiles).

Per layer: LN1 -> K,V projections per head-group (4 heads) -> AllGather
kv (fp8) per group (2 smaller collectives overlap with Q projection and
the first group's attention) -> attention with variable-width causal
slabs -> out-projection -> LN2 -> MLP (bf16) -> residual.

Precision: residual fp32 (+bf16 shadow for LN sums), QKV/proj matmuls
fp8e4 DoubleRow (2 k-tiles per MM), attention fp8 (scores/AV normal
mode), MLP and head bf16 (fp8 there fails the 2e-2 gate: no softmax
averaging to absorb activation-quantization noise).

Softmax: exp on scalar engine with fused per-partition column-disable
bias; causal masking only on the first 128-col slab of each position via
per-core mask data (triangle/ones/zeros); denominators via
reciprocal_approx_fast + PE-broadcast (no DRAM round-trips).
"""

import os
import sys

import numpy as np

for _p in ("/opt/trn_rl_repo",):
    if _p not in sys.path and os.path.isdir(_p):
        sys.path.insert(0, _p)

import ml_dtypes  # noqa: E402

BF16 = ml_dtypes.bfloat16
FP8 = ml_dtypes.float8_e4m3

# model dims
B, T, C, L = 4, 1024, 1024, 4
NH, HD = 8, 128
HID = 4 * C
V1 = 101  # V + 1
TD = 64  # TOTAL_DIM
NUM_NODE, F_DIM = 15, 4
D_BIAS = NUM_NODE * F_DIM  # 60
EPS = 1e-5
NCORES = 8
TOK = T // 2  # 512 tokens per core
KSUB = C // 128  # 8
HSUB = HID // 128  # 32
SQHD = float(np.sqrt(HD))  # 11.31

WS = 16.0  # fp8 weight prescale (folded back after matmul)

# token-tile assignment: even cores set A, odd cores set B
TILES_A = [1, 2, 4, 7]
TILES_B = [0, 3, 5, 6]
# gathered k-tile order = [A tiles; B tiles]
GATH = TILES_A + TILES_B
# per-position slab start (cols S..512 computed at position p)
SLAB = [0, 128, 256, 384, 0, 128, 256, 384]

_CACHED = {}


def _build_program(fp8=True):
    import concourse.bacc as bacc
    import concourse.bass as bass  # noqa: F401
    import concourse.mybir as mybir
    import concourse.tile as tile

    dt = mybir.dt
    AF = mybir.ActivationFunctionType
    OP = mybir.AluOpType
    DR = mybir.MatmulPerfMode.DoubleRow if fp8 else None
    f8 = dt.float8e4 if fp8 else dt.bfloat16

    nc = bacc.Bacc(trn_type="TRN2", num_devices=NCORES)

    # ---- I/O ----
    h0T_d = nc.dram_tensor("h0T", (128, KSUB, TOK), dt.bfloat16, kind="ExternalInput")
    dm_d = nc.dram_tensor("dm", (128, 8, 128), dt.bfloat16, kind="ExternalInput")
    pbT_d = nc.dram_tensor("pbT", (64, NH, 64), dt.float32, kind="ExternalInput")
    cb_d = nc.dram_tensor("cb", (128, 1), dt.float32, kind="ExternalInput")
    wq_d = nc.dram_tensor("wq", (L, 128, KSUB, C), f8, kind="ExternalInput")
    wk_d = nc.dram_tensor("wk", (L, 128, KSUB, C), f8, kind="ExternalInput")
    wv_d = nc.dram_tensor("wv", (L, 128, KSUB, C), f8, kind="ExternalInput")
    wp_d = nc.dram_tensor("wp", (L, 128, KSUB, C), f8, kind="ExternalInput")
    w1_d = nc.dram_tensor("w1", (L, 8, 128, KSUB, 512), dt.bfloat16, kind="ExternalInput")
    w2_d = nc.dram_tensor("w2", (L, 4, HSUB, 128, 256), dt.bfloat16, kind="ExternalInput")
    hwT_d = nc.dram_tensor("hwT", (TD, 128, KSUB, 128), dt.bfloat16, kind="ExternalInput")
    out_d = nc.dram_tensor("logits", (TD, TOK // TD, V1), dt.float32, kind="ExternalOutput")

    RG = [[0, 1], [2, 3], [4, 5], [6, 7]]
    KVQ = 4 * HD * TOK  # elements per k (or v) group shard

    with tile.TileContext(nc) as tc:
        consts = tc.alloc_tile_pool(name="consts", bufs=1)
        hpool = tc.alloc_tile_pool(name="hpool", bufs=1)
        wpool = tc.alloc_tile_pool(name="wpool", bufs=2)
        w1pool = tc.alloc_tile_pool(name="w1pool", bufs=2)
        w2pool = tc.alloc_tile_pool(name="w2pool", bufs=4)
        apool = tc.alloc_tile_pool(name="apool", bufs=2)
        dpool = tc.alloc_tile_pool(name="dpool", bufs=2, space="DRAM")
        # PSUM: acc 4 banks (persistent accumulators), s 2 (transient mm),
        # v 2 (reductions/broadcasts) = 8 banks
        ps_acc = tc.alloc_tile_pool(name="ps_acc", bufs=2, space="PSUM")
        ps_s = tc.alloc_tile_pool(name="ps_s", bufs=4, space="PSUM")
        ps_v = tc.alloc_tile_pool(name="ps_v", bufs=2, space="PSUM")

        # ---- constants ----
        dm_sb = consts.tile([128, 8, 128], dt.bfloat16, name="dm_sb")
        nc.sync.dma_start(dm_sb, dm_d[:])
        pb_sb = consts.tile([64, NH, 64], dt.float32, name="pb_sb")
        nc.sync.dma_start(pb_sb, pbT_d[:])
        cb_sb = consts.tile([128, 1], dt.float32, name="cb_sb")
        nc.sync.dma_start(cb_sb, cb_d[:])
        ones8 = consts.tile([128, 1], f8, name="ones8")
        nc.vector.memset(ones8, 1.0)
        ones16 = consts.tile([128, 1], dt.bfloat16, name="ones16")
        nc.vector.memset(ones16, 1.0)
        onesrow = consts.tile([1, 128], dt.bfloat16, name="onesrow")
        nc.vector.memset(onesrow, 1.0)

        # ---- residual stream: bf16 ----
        h16 = hpool.tile([128, KSUB, TOK], dt.bfloat16, name="h16")
        nc.sync.dma_start(h16, h0T_d[:])

        def layernorm(dst, dst_dt_scale=None):
            """LN over C of h16 -> dst.  Stats bf16, sums via PE."""
            sum_ps = ps_v.tile([1, TOK], dt.float32, tag="v", name="sum_ps")
            for ct in range(KSUB):
                nc.tensor.matmul(
                    sum_ps, ones16, h16[:, ct, :],
                    start=(ct == 0), stop=(ct == KSUB - 1),
                )
            ssq_ps = ps_v.tile([1, TOK], dt.float32, tag="v", name="ssq_ps")
            for ct in range(KSUB):
                sq16 = apool.tile([128, TOK], dt.bfloat16, tag="sq", name="sq16")
                nc.vector.tensor_mul(sq16, h16[:, ct, :], h16[:, ct, :])
                nc.tensor.matmul(
                    ssq_ps, ones16, sq16,
                    start=(ct == 0), stop=(ct == KSUB - 1),
                )
            stat = apool.tile([1, 3 * TOK], dt.float32, tag="stat", name="stat")
            mean = stat[:, 0:TOK]
            var = stat[:, TOK : 2 * TOK]
            rvar = stat[:, 2 * TOK : 3 * TOK]
            nc.vector.tensor_scalar_mul(mean, sum_ps, 1.0 / C)
            msq = apool.tile([1, TOK], dt.float32, tag="msq", name="msq")
            nc.vector.tensor_mul(msq, mean, mean)
            # var = ssq/C - mean^2 + eps
            nc.vector.scalar_tensor_tensor(
                var, ssq_ps, 1.0 / C, msq, OP.mult, OP.subtract
            )
            nc.vector.tensor_scalar_add(var, var, EPS)
            nc.vector.reciprocal_approx_fast(rvar, var)
            st16 = apool.tile([1, 2 * TOK], dt.bfloat16, tag="st16", name="st16")
            rstd16 = st16[:, 0:TOK]
            mr16 = st16[:, TOK : 2 * TOK]
            nc.scalar.activation(rstd16, rvar, AF.Sqrt)  # 1/sqrt(var)
            nc.vector.tensor_mul(mr16, mean, rstd16)
            # broadcast via PE (K=1 matmul)
            rstd_b = ps_v.tile([128, TOK], dt.float32, tag="v", name="rstd_b")
            nc.tensor.matmul(rstd_b, onesrow, rstd16, start=True, stop=True)
            mr_b = ps_v.tile([128, TOK], dt.float32, tag="v", name="mr_b")
            nc.tensor.matmul(mr_b, onesrow, mr16, start=True, stop=True)
            for ct in range(KSUB):
                tmp = apool.tile([128, TOK], dt.float32, tag="lntmp", name="lntmp")
                nc.vector.tensor_mul(tmp, h16[:, ct, :], rstd_b)
                nc.vector.tensor_sub(dst[:, ct, :], tmp, mr_b)

        def resid_add(co, pd, scale):
            """h16[:, co] += pd*scale (in place)."""
            if scale != 1.0:
                nc.vector.scalar_tensor_tensor(
                    h16[:, co, :], pd, scale, h16[:, co, :], OP.mult, OP.add
                )
            else:
                nc.vector.tensor_add(h16[:, co, :], h16[:, co, :], pd)

        IWS = (1.0 / WS) if fp8 else 1.0

        for layer in range(L):
            # ---------- LN1 ----------
            aT = apool.tile([128, KSUB, TOK], f8, tag="aT", name="aT")
            layernorm(aT)

            # ---------- K, V projections per group + AllGather ----------
            wk_sb = wpool.tile([128, KSUB, C], f8, tag="wmat", name="wk_sb")
            nc.sync.dma_start(wk_sb, wk_d[layer])
            wv_sb = wpool.tile([128, KSUB, C], f8, tag="wmat", name="wv_sb")
            nc.sync.dma_start(wv_sb, wv_d[layer])

            kv_ga = []
            for grp in range(2):
                kv_in = dpool.tile([2, KVQ], f8, tag="kvin", name="kv_in")
                kv_g = dpool.tile([2, 2, KVQ], f8, tag="kvga", name="kv_ga")
                kin = kv_in[0].rearrange("(c t) -> c t", t=TOK)  # (512, TOK) head-major
                vin = kv_in[1].rearrange("(t c) -> t c", c=4 * HD)  # (TOK, 512) tok-major
                for hl in range(4):
                    hh = grp * 4 + hl
                    pk = ps_s.tile([128, TOK], dt.float32, tag="s", name="pk")
                    if fp8:
                        for gp in range(4):
                            nc.tensor.matmul(
                                pk,
                                wk_sb[:, 2 * gp : 2 * gp + 2, hh * HD : (hh + 1) * HD],
                                aT[:, 2 * gp : 2 * gp + 2, :],
                                start=(gp == 0), stop=(gp == 3), perf_mode=DR,
                            )
                    else:
                        for ct in range(KSUB):
                            nc.tensor.matmul(
                                pk, wk_sb[:, ct, hh * HD : (hh + 1) * HD],
                                aT[:, ct, :],
                                start=(ct == 0), stop=(ct == KSUB - 1),
                            )
                    k8 = apool.tile([128, TOK], f8, tag="kv8", name="k8")
                    nc.vector.tensor_scalar_mul(k8, pk, IWS)
                    nc.sync.dma_start(kin[hl * HD : (hl + 1) * HD, :], k8)
                for tsub in range(4):
                    pv = ps_s.tile([128, 512], dt.float32, tag="s", name="pv")
                    if fp8:
                        for gp in range(4):
                            nc.tensor.matmul(
                                pv,
                                aT[:, 2 * gp : 2 * gp + 2, tsub * 128 : (tsub + 1) * 128],
                                wv_sb[:, 2 * gp : 2 * gp + 2, grp * 512 : (grp + 1) * 512],
                                start=(gp == 0), stop=(gp == 3), perf_mode=DR,
                            )
                    else:
                        for ct in range(KSUB):
                            nc.tensor.matmul(
                                pv,
                                aT[:, ct, tsub * 128 : (tsub + 1) * 128],
                                wv_sb[:, ct, grp * 512 : (grp + 1) * 512],
                                start=(ct == 0), stop=(ct == KSUB - 1),
                            )
                    v8 = apool.tile([128, 512], f8, tag="kv8", name="v8")
                    nc.vector.tensor_scalar_mul(v8, pv, IWS)
                    nc.sync.dma_start(vin[tsub * 128 : (tsub + 1) * 128, :], v8)
                nc.gpsimd.collective_compute(
                    "AllGather",
                    OP.bypass,
                    replica_groups=RG,
                    ins=[kv_in.opt()],
                    outs=[kv_g.opt()],
                )
                kv_ga.append(kv_g)

            # ---------- Q projection (overlaps AllGathers) ----------
            wq_sb = wpool.tile([128, KSUB, C], f8, tag="wmat", name="wq_sb")
            nc.sync.dma_start(wq_sb, wq_d[layer])
            qT = apool.tile([128, NH, TOK], f8, tag="qT", name="qT", bufs=1)
            for hh in range(NH):
                pq = ps_s.tile([128, TOK], dt.float32, tag="s", name="pq")
                if fp8:
                    for gp in range(4):
                        nc.tensor.matmul(
                            pq,
                            wq_sb[:, 2 * gp : 2 * gp + 2, hh * HD : (hh + 1) * HD],
                            aT[:, 2 * gp : 2 * gp + 2, :],
                            start=(gp == 0), stop=(gp == 3), perf_mode=DR,
                        )
                else:
                    for ct in range(KSUB):
                        nc.tensor.matmul(
                            pq, wq_sb[:, ct, hh * HD : (hh + 1) * HD], aT[:, ct, :],
                            start=(ct == 0), stop=(ct == KSUB - 1),
                        )
                nc.vector.tensor_scalar_mul(qT[:, hh, :], pq, IWS)

            # ---------- attention ----------
            yT = apool.tile([128, NH, TOK], f8, tag="yT", name="yT", bufs=1)
            for grp in range(2):
                kv_g = kv_ga[grp]
                kg_all = apool.tile([128, 2, 4, TOK], f8, tag="kg", name="kg_all")
                vg_all = apool.tile([128, 8, 512], f8, tag="vg", name="vg_all")
                for r in range(2):
                    kga = kv_g[r, 0].rearrange("(hl hd t) -> hd hl t", hd=HD, t=TOK)
                    vga = kv_g[r, 1].rearrange("(ts t c) -> t ts c", t=128, c=4 * HD)
                    nc.sync.dma_start(kg_all[:, r, :, :], kga)
                    nc.sync.dma_start(vg_all[:, r * 4 : (r + 1) * 4, :], vga)
                for hl in range(4):
                    hh = grp * 4 + hl

                    e_sb = apool.tile([128, 8, TOK], f8, tag="e_sb", name="e_sb", bufs=3)
                    den_ps = ps_v.tile([1, TOK], dt.float32, tag="v", name="den_ps")
                    py = ps_acc.tile([128, TOK], dt.float32, tag="acc", name="py")

                    def den_av(p):
                        # den/AV accumulation, deferred 3 positions behind the
                        # scores so the PE never blocks on exp (FIFO queues).
                        # AV: odd p pairs with p-1 in DoubleRow over the
                        # narrower suffix; the 128-col head of the even
                        # position is done in normal mode.
                        S = SLAB[p]
                        nc.tensor.matmul(
                            den_ps[:, S:TOK], ones8, e_sb[:, p, S:TOK],
                            start=(p == 0), stop=(p == 7),
                        )
                        nc.tensor.matmul(
                            py[:, S:TOK],
                            vg_all[:, p, hl * HD : (hl + 1) * HD],
                            e_sb[:, p, S:TOK],
                            start=(p == 0), stop=(p == 7),
                        )

                    for p in range(8):
                        S = SLAB[p]
                        s_ps = ps_s.tile([128, TOK], dt.float32, tag="s", name="s_ps")
                        nc.tensor.matmul(
                            s_ps[:, S:TOK],
                            kg_all[:, p // 4, hl, (p % 4) * 128 : (p % 4 + 1) * 128],
                            qT[:, hh, S:TOK],
                            start=True, stop=True,
                        )
                        if p == 4:
                            nc.vector.tensor_add(
                                s_ps[0:64, 0:64], s_ps[0:64, 0:64], pb_sb[:, hh, :]
                            )
                        nc.scalar.activation(
                            e_sb[:, p, S:TOK], s_ps[:, S:TOK], AF.Exp,
                            bias=cb_sb, scale=1.0 / SQHD,
                        )
                        nc.vector.tensor_mul(
                            e_sb[:, p, S : S + 128],
                            e_sb[:, p, S : S + 128],
                            dm_sb[:, p, :],
                        )
                        if p >= 3:
                            den_av(p - 3)
                    for p in range(5, 8):
                        den_av(p)
                    den16 = apool.tile([1, TOK], dt.bfloat16, tag="den16", name="den16")
                    nc.vector.tensor_copy(den16, den_ps)
                    den_b = ps_v.tile([128, TOK], dt.float32, tag="v", name="den_b")
                    nc.tensor.matmul(den_b, onesrow, den16, start=True, stop=True)
                    recB = apool.tile([128, TOK], dt.float32, tag="recB", name="recB")
                    nc.vector.reciprocal_approx_fast(recB, den_b)
                    nc.vector.tensor_mul(yT[:, hh, :], py, recB)

            # ---------- proj + residual ----------
            wp_sb = wpool.tile([128, KSUB, C], f8, tag="wmat", name="wp_sb")
            nc.sync.dma_start(wp_sb, wp_d[layer])
            for co in range(KSUB):
                pp = ps_s.tile([128, TOK], dt.float32, tag="s", name="pp")
                if fp8:
                    for gp in range(4):
                        nc.tensor.matmul(
                            pp,
                            wp_sb[:, 2 * gp : 2 * gp + 2, co * 128 : (co + 1) * 128],
                            yT[:, 2 * gp : 2 * gp + 2, :],
                            start=(gp == 0), stop=(gp == 3), perf_mode=DR,
                        )
                else:
                    for ct in range(KSUB):
                        nc.tensor.matmul(
                            pp, wp_sb[:, ct, co * 128 : (co + 1) * 128], yT[:, ct, :],
                            start=(ct == 0), stop=(ct == KSUB - 1),
                        )
                resid_add(co, pp, IWS)

            # ---------- LN2 ----------
            a2T = apool.tile([128, KSUB, TOK], dt.bfloat16, tag="aT", name="a2T")
            layernorm(a2T)

            # ---------- MLP (bf16) ----------
            g_sb = apool.tile([128, HSUB, TOK], dt.bfloat16, tag="g_sb", name="g_sb",
                              bufs=1)
            for hblk in range(8):
                w1_sb = w1pool.tile([128, KSUB, 512], dt.bfloat16, tag="w1b",
                                    name="w1_sb")
                nc.sync.dma_start(w1_sb, w1_d[layer, hblk])
                for hc in range(4):
                    pu = ps_s.tile([128, TOK], dt.float32, tag="s", name="pu")
                    for ct in range(KSUB):
                        nc.tensor.matmul(
                            pu,
                            w1_sb[:, ct, hc * 128 : (hc + 1) * 128],
                            a2T[:, ct, :],
                            start=(ct == 0), stop=(ct == KSUB - 1),
                        )
                    nc.scalar.activation(g_sb[:, hblk * 4 + hc, :], pu, AF.Gelu)

            for grp2 in range(4):
                pd = [
                    ps_acc.tile([128, TOK], dt.float32, tag="acc", name=f"pd{i}")
                    for i in range(2)
                ]
                for ksub in range(HSUB):
                    w2_sb = w2pool.tile([128, 256], dt.bfloat16, tag="w2t",
                                        name="w2_sb")
                    nc.sync.dma_start(w2_sb, w2_d[layer, grp2, ksub])
                    for i in range(2):
                        nc.tensor.matmul(
                            pd[i],
                            w2_sb[:, i * 128 : (i + 1) * 128],
                            g_sb[:, ksub, :],
                            start=(ksub == 0), stop=(ksub == HSUB - 1),
                        )
                for i in range(2):
                    resid_add(grp2 * 2 + i, pd[i], 1.0)

        # ---------- final LN + head (bf16) ----------
        hfT = apool.tile([128, KSUB, TOK], dt.bfloat16, tag="aT", name="hfT")
        layernorm(hfT)
        hfT_r = hfT.rearrange("p k (b e) -> p k e b", e=TD)  # b: 8 blocks of 64
        NB = TOK // TD  # 8 tokens per class
        for eg in range(TD // 4):  # 4 e-classes concurrently via PE col groups
            hw4 = [
                w1pool.tile([128, KSUB, 128], dt.bfloat16, tag=f"hw{j}",
                            name=f"hw4_{j}", bufs=2)
                for j in range(4)
            ]
            for j in range(4):
                nc.sync.dma_start(hw4[j], hwT_d[4 * eg + j])
            po4 = ps_s.tile([128, V1], dt.float32, tag="s", name="po4")
            for ct in range(KSUB):
                for j in range(4):
                    nc.tensor.matmul(
                        po4[32 * j : 32 * j + NB, :],
                        hfT_r[:, ct, 4 * eg + j, :],
                        hw4[j][:, ct, 0:V1],
                        start=(ct == 0), stop=(ct == KSUB - 1),
                        tile_position=(0, 32 * j),
                    )
            o_sb = apool.tile([128, V1], dt.float32, tag="o_sb", name="o_sb")
            nc.vector.tensor_copy(o_sb, po4)
            for j in range(4):
                nc.sync.dma_start(out_d[4 * eg + j], o_sb[32 * j : 32 * j + NB, :])

        for p in (ps_v, ps_s, ps_acc, dpool, apool, w2pool, w1pool, wpool, hpool,
                  consts):
            p.release()

    nc.compile()
    return nc


def _tok_idx(half):
    tiles = TILES_A if half == 0 else TILES_B
    return np.concatenate([np.arange(t * 128, (t + 1) * 128) for t in tiles])


def _host_inputs(x, attn_bias, pos_emb, Wq, Wk, Wv, Wp, w1, w2, head_w, fp8=True):
    f8c = (lambda a: np.clip(a, -240, 240).astype(FP8)) if fp8 else (
        lambda a: a.astype(BF16))

    def packw(W):
        # (L, C, N) -> (L, 128, KSUB, N): partition-major contiguous
        W = np.asarray(W, np.float32)
        n = W.shape[2]
        return np.ascontiguousarray(
            W.reshape(L, KSUB, 128, n).transpose(0, 2, 1, 3))

    sc = WS if fp8 else 1.0
    wq = f8c(packw(Wq) * sc)
    wk = f8c(packw(Wk) * sc)
    wv = f8c(packw(Wv) * sc)
    wp = f8c(packw(Wp) * sc)
    w1b = np.ascontiguousarray(
        np.asarray(w1, np.float32).reshape(L, KSUB, 128, 8, 512)
        .transpose(0, 3, 2, 1, 4)).astype(BF16)
    w2b = np.ascontiguousarray(
        np.asarray(w2, np.float32).reshape(L, HSUB, 128, 4, 256)
        .transpose(0, 3, 1, 2, 4)).astype(BF16)
    hwT = np.zeros((TD, C, 128), np.float32)
    hwT[:, :, :V1] = np.asarray(head_w, np.float32).transpose(0, 2, 1)
    hwT = np.ascontiguousarray(
        hwT.reshape(TD, KSUB, 128, 128).transpose(0, 2, 1, 3)).astype(BF16)

    # graph bias, transposed (kv, head, q), padded 60->64, prescaled by
    # sqrt(HD) (the kernel applies exp(s/sqrt(HD) + colbias))
    bias = np.repeat(np.repeat(np.asarray(attn_bias, np.float32), F_DIM, 1), F_DIM, 2)
    pbT = np.zeros((64, NH, 64), np.float32)
    pbT[:D_BIAS, :, :D_BIAS] = bias.transpose(2, 0, 1) * SQHD  # [j, h, i]
    pbT_zero = np.zeros_like(pbT)

    # column-disable bias: kv rows with global_j % 64 == 63 get -50
    cb = np.zeros((128, 1), np.float32)
    cb[63, 0] = -50.0
    cb[127, 0] = -50.0

    # per-position first-slab masks [128(k), 8(pos), 128(q)]
    tri = np.tril(np.ones((128, 128), np.float32)).T  # tri[k, q] = k <= q
    onesm = np.ones((128, 128), np.float32)
    zerom = np.zeros((128, 128), np.float32)
    dm_A = np.stack([tri, tri, tri, tri, onesm, zerom, zerom, onesm], 1)
    dm_B = np.stack([zerom, onesm, onesm, zerom, tri, tri, tri, tri], 1)
    dm_A = np.ascontiguousarray(dm_A).astype(BF16)
    dm_B = np.ascontiguousarray(dm_B).astype(BF16)

    h0 = np.asarray(x, np.float32) + np.asarray(pos_emb, np.float32)  # (B, T, C)

    in_maps = []
    for core in range(NCORES):
        b, half = core // 2, core % 2
        idx = _tok_idx(half)
        h0T = np.ascontiguousarray(
            h0[b, idx].T.reshape(KSUB, 128, TOK).transpose(1, 0, 2)
        ).astype(BF16)  # (128, KSUB, TOK)
        in_maps.append(
            {
                "h0T": h0T,
                "dm": dm_A if half == 0 else dm_B,
                "pbT": pbT_zero if half == 0 else pbT,
                "cb": cb,
                "wq": wq, "wk": wk, "wv": wv, "wp": wp,
                "w1": w1b, "w2": w2b, "hwT": hwT,
            }
        )
    return in_maps


def kernel(**inputs):
    from concourse.bass_utils import run_bass_kernel_spmd

    fp8 = bool(int(os.environ.get("KERNEL_FP8", "1")))
    in_maps = _host_inputs(
        inputs["x"], inputs["attn_bias"], inputs["pos_emb"],
        inputs["Wq"], inputs["Wk"], inputs["Wv"], inputs["Wp"],
        inputs["w1"], inputs["w2"], inputs["head_w"], fp8=fp8,
    )
    if "nc" not in _CACHED:
        _CACHED["nc"] = _build_program(fp8=fp8)
    res = run_bass_kernel_spmd(
        _CACHED["nc"], in_maps, core_ids=list(range(NCORES)),
        trace=bool(int(os.environ.get("KERNEL_TRACE", "0"))),
    )
    out = np.zeros((B, T, V1), np.float32)
    for core in range(NCORES):
        b, half = core // 2, core % 2
        lg = res.results[core]["logits"]  # (TD, 8, V1): token = b*64 + e
        lg = lg.transpose(1, 0, 2).reshape(TOK, V1)
        out[b, _tok_idx(half)] = lg
    _CACHED["last_result"] = res
    return out


# revision 12
# speedup vs baseline: 1.1739x; 1.1654x over previous
"""GPT (4-layer, B=4 T=1024 C=1024 NH=8) Trainium2 Bass kernel, v2.

Sharding: 8 cores = 4 batches (DP) x 2 interleaved token-tile sets (SP).
Even cores own token tiles {1,2,4,7} of their batch, odd cores {0,3,5,6}
(balanced causal work: each set needs 18 of 36 causal 128x128 tiles).

Per layer: LN1 -> K,V projections per head-group (4 heads) -> AllGather
kv (fp8) per group (2 smaller collectives overlap with Q projection and
the first group's attention) -> attention with variable-width causal
slabs -> out-projection -> LN2 -> MLP (bf16) -> residual.

Precision: residual fp32 (+bf16 shadow for LN sums), QKV/proj matmuls
fp8e4 DoubleRow (2 k-tiles per MM), attention fp8 (scores/AV normal
mode), MLP and head bf16 (fp8 there fails the 2e-2 gate: no softmax
averaging to absorb activation-quantization noise).

Softmax: exp on scalar engine with fused per-partition column-disable
bias; causal masking only on the first 128-col slab of each position via
per-core mask data (triangle/ones/zeros); denominators via
reciprocal_approx_fast + PE-broadcast (no DRAM round-trips).
"""

import os
import sys

import numpy as np

for _p in ("/opt/trn_rl_repo",):
    if _p not in sys.path and os.path.isdir(_p):
        sys.path.insert(0, _p)

import ml_dtypes  # noqa: E402

BF16 = ml_dtypes.bfloat16
FP8 = ml_dtypes.float8_e4m3

# model dims
B, T, C, L = 4, 1024, 1024, 4
NH, HD = 8, 128
HID = 4 * C
V1 = 101  # V + 1
TD = 64  # TOTAL_DIM
NUM_NODE, F_DIM = 15, 4
D_BIAS = NUM_NODE * F_DIM  # 60
EPS = 1e-5
NCORES = 8
TOK = T // 2  # 512 tokens per core
KSUB = C // 128  # 8
HSUB = HID // 128  # 32
SQHD = float(np.sqrt(HD))  # 11.31

WS = 16.0  # fp8 weight prescale (folded back after matmul)

# token-tile assignment: even cores set A, odd cores set B
TILES_A = [1, 2, 4, 7]
TILES_B = [0, 3, 5, 6]
# gathered k-tile order = [A tiles; B tiles]
GATH = TILES_A + TILES_B
# per-position slab start (cols S..512 computed at position p)
SLAB = [0, 128, 256, 384, 0, 128, 256, 384]

_CACHED = {}


def _build_program(fp8=True):
    import concourse.bacc as bacc
    import concourse.bass as bass  # noqa: F401
    import concourse.mybir as mybir
    import concourse.tile as tile

    dt = mybir.dt
    AF = mybir.ActivationFunctionType
    OP = mybir.AluOpType
    DR = mybir.MatmulPerfMode.DoubleRow if fp8 else None
    f8 = dt.float8e4 if fp8 else dt.bfloat16

    nc = bacc.Bacc(trn_type="TRN2", num_devices=NCORES)

    # ---- I/O ----
    h0T_d = nc.dram_tensor("h0T", (128, KSUB, TOK), dt.bfloat16, kind="ExternalInput")
    dm_d = nc.dram_tensor("dm", (128, 8, 128), dt.bfloat16, kind="ExternalInput")
    pbT_d = nc.dram_tensor("pbT", (64, NH, 64), dt.float32, kind="ExternalInput")
    cb_d = nc.dram_tensor("cb", (128, 1), dt.float32, kind="ExternalInput")
    wq_d = nc.dram_tensor("wq", (L, 128, KSUB, C), f8, kind="ExternalInput")
    wk_d = nc.dram_tensor("wk", (L, 128, KSUB, C), f8, kind="ExternalInput")
    wv_d = nc.dram_tensor("wv", (L, 128, KSUB, C), f8, kind="ExternalInput")
    wp_d = nc.dram_tensor("wp", (L, 128, KSUB, C), f8, kind="ExternalInput")
    w1_d = nc.dram_tensor("w1", (L, 8, 128, KSUB, 512), dt.bfloat16, kind="ExternalInput")
    w2_d = nc.dram_tensor("w2", (L, 4, HSUB, 128, 256), dt.bfloat16, kind="ExternalInput")
    hwT_d = nc.dram_tensor("hwT", (TD, 128, KSUB, 128), dt.bfloat16, kind="ExternalInput")
    out_d = nc.dram_tensor("logits", (TD, TOK // TD, V1), dt.float32, kind="ExternalOutput")

    RG = [[0, 1], [2, 3], [4, 5], [6, 7]]
    KVQ = 4 * HD * TOK  # elements per k (or v) group shard

    with tile.TileContext(nc) as tc:
        consts = tc.alloc_tile_pool(name="consts", bufs=1)
        hpool = tc.alloc_tile_pool(name="hpool", bufs=1)
        wpool = tc.alloc_tile_pool(name="wpool", bufs=2)
        w1pool = tc.alloc_tile_pool(name="w1pool", bufs=2)
        w2pool = tc.alloc_tile_pool(name="w2pool", bufs=4)
        apool = tc.alloc_tile_pool(name="apool", bufs=2)
        dpool = tc.alloc_tile_pool(name="dpool", bufs=2, space="DRAM")
        # PSUM: acc 4 banks (persistent accumulators), s 2 (transient mm),
        # v 2 (reductions/broadcasts) = 8 banks
        ps_acc = tc.alloc_tile_pool(name="ps_acc", bufs=2, space="PSUM")
        ps_s = tc.alloc_tile_pool(name="ps_s", bufs=4, space="PSUM")
        ps_v = tc.alloc_tile_pool(name="ps_v", bufs=2, space="PSUM")

        # ---- constants ----
        dm_sb = consts.tile([128, 8, 128], dt.bfloat16, name="dm_sb")
        nc.sync.dma_start(dm_sb, dm_d[:])
        pb_sb = consts.tile([64, NH, 64], dt.float32, name="pb_sb")
        nc.sync.dma_start(pb_sb, pbT_d[:])
        cb_sb = consts.tile([128, 1], dt.float32, name="cb_sb")
        nc.sync.dma_start(cb_sb, cb_d[:])
        ones8 = consts.tile([128, 1], f8, name="ones8")
        nc.vector.memset(ones8, 1.0)
        ones16 = consts.tile([128, 1], dt.bfloat16, name="ones16")
        nc.vector.memset(ones16, 1.0)
        onesrow = consts.tile([1, 128], dt.bfloat16, name="onesrow")
        nc.vector.memset(onesrow, 1.0)

        # ---- residual stream: bf16 ----
        h16 = hpool.tile([128, KSUB, TOK], dt.bfloat16, name="h16")
        nc.sync.dma_start(h16, h0T_d[:])

        def layernorm(dst, dst_dt_scale=None):
            """LN over C of h16 -> dst.  Stats bf16, sums via PE."""
            sum_ps = ps_v.tile([1, TOK], dt.float32, tag="v", name="sum_ps")
            for ct in range(KSUB):
                nc.tensor.matmul(
                    sum_ps, ones16, h16[:, ct, :],
                    start=(ct == 0), stop=(ct == KSUB - 1),
                )
            ssq_ps = ps_v.tile([1, TOK], dt.float32, tag="v", name="ssq_ps")
            for ct in range(KSUB):
                sq16 = apool.tile([128, TOK], dt.bfloat16, tag="sq", name="sq16")
                nc.vector.tensor_mul(sq16, h16[:, ct, :], h16[:, ct, :])
                nc.tensor.matmul(
                    ssq_ps, ones16, sq16,
                    start=(ct == 0), stop=(ct == KSUB - 1),
                )
            stat = apool.tile([1, 3 * TOK], dt.float32, tag="stat", name="stat")
            mean = stat[:, 0:TOK]
            var = stat[:, TOK : 2 * TOK]
            rvar = stat[:, 2 * TOK : 3 * TOK]
            nc.vector.tensor_scalar_mul(mean, sum_ps, 1.0 / C)
            msq = apool.tile([1, TOK], dt.float32, tag="msq", name="msq")
            nc.vector.tensor_mul(msq, mean, mean)
            # var = ssq/C - mean^2 + eps
            nc.vector.scalar_tensor_tensor(
                var, ssq_ps, 1.0 / C, msq, OP.mult, OP.subtract
            )
            nc.vector.tensor_scalar_add(var, var, EPS)
            nc.vector.reciprocal_approx_fast(rvar, var)
            st16 = apool.tile([1, 2 * TOK], dt.bfloat16, tag="st16", name="st16")
            rstd16 = st16[:, 0:TOK]
            mr16 = st16[:, TOK : 2 * TOK]
            nc.scalar.activation(rstd16, rvar, AF.Sqrt)  # 1/sqrt(var)
            nc.vector.tensor_mul(mr16, mean, rstd16)
            # broadcast via PE (K=1 matmul)
            rstd_b = ps_v.tile([128, TOK], dt.float32, tag="v", name="rstd_b")
            nc.tensor.matmul(rstd_b, onesrow, rstd16, start=True, stop=True)
            mr_b = ps_v.tile([128, TOK], dt.float32, tag="v", name="mr_b")
            nc.tensor.matmul(mr_b, onesrow, mr16, start=True, stop=True)
            for ct in range(KSUB):
                tmp = apool.tile([128, TOK], dt.float32, tag="lntmp", name="lntmp")
                nc.vector.tensor_mul(tmp, h16[:, ct, :], rstd_b)
                nc.vector.tensor_sub(dst[:, ct, :], tmp, mr_b)

        def resid_add(co, pd, scale):
            """h16[:, co] += pd*scale (in place)."""
            if scale != 1.0:
                nc.vector.scalar_tensor_tensor(
                    h16[:, co, :], pd, scale, h16[:, co, :], OP.mult, OP.add
                )
            else:
                nc.vector.tensor_add(h16[:, co, :], h16[:, co, :], pd)

        IWS = (1.0 / WS) if fp8 else 1.0

        for layer in range(L):
            # ---------- LN1 ----------
            aT = apool.tile([128, KSUB, TOK], f8, tag="aT", name="aT")
            layernorm(aT)

            # ---------- K, V projections per group + AllGather ----------
            wk_sb = wpool.tile([128, KSUB, C], f8, tag="wmat", name="wk_sb")
            nc.sync.dma_start(wk_sb, wk_d[layer])
            wv_sb = wpool.tile([128, KSUB, C], f8, tag="wmat", name="wv_sb")
            nc.sync.dma_start(wv_sb, wv_d[layer])

            kv_ga = []
            for grp in range(2):
                kv_in = dpool.tile([2, KVQ], f8, tag="kvin", name="kv_in")
                kv_g = dpool.tile([2, 2, KVQ], f8, tag="kvga", name="kv_ga")
                kin = kv_in[0].rearrange("(c t) -> c t", t=TOK)  # (512, TOK) head-major
                vin = kv_in[1].rearrange("(t c) -> t c", c=4 * HD)  # (TOK, 512) tok-major
                for hl in range(4):
                    hh = grp * 4 + hl
                    pk = ps_s.tile([128, TOK], dt.float32, tag="s", name="pk")
                    if fp8:
                        for gp in range(4):
                            nc.tensor.matmul(
                                pk,
                                wk_sb[:, 2 * gp : 2 * gp + 2, hh * HD : (hh + 1) * HD],
                                aT[:, 2 * gp : 2 * gp + 2, :],
                                start=(gp == 0), stop=(gp == 3), perf_mode=DR,
                            )
                    else:
                        for ct in range(KSUB):
                            nc.tensor.matmul(
                                pk, wk_sb[:, ct, hh * HD : (hh + 1) * HD],
                                aT[:, ct, :],
                                start=(ct == 0), stop=(ct == KSUB - 1),
                            )
                    k8 = apool.tile([128, TOK], f8, tag="kv8", name="k8")
                    nc.vector.tensor_scalar_mul(k8, pk, IWS)
                    nc.sync.dma_start(kin[hl * HD : (hl + 1) * HD, :], k8)
                for tsub in range(4):
                    pv = ps_s.tile([128, 512], dt.float32, tag="s", name="pv")
                    if fp8:
                        for gp in range(4):
                            nc.tensor.matmul(
                                pv,
                                aT[:, 2 * gp : 2 * gp + 2, tsub * 128 : (tsub + 1) * 128],
                                wv_sb[:, 2 * gp : 2 * gp + 2, grp * 512 : (grp + 1) * 512],
                                start=(gp == 0), stop=(gp == 3), perf_mode=DR,
                            )
                    else:
                        for ct in range(KSUB):
                            nc.tensor.matmul(
                                pv,
                                aT[:, ct, tsub * 128 : (tsub + 1) * 128],
                                wv_sb[:, ct, grp * 512 : (grp + 1) * 512],
                                start=(ct == 0), stop=(ct == KSUB - 1),
                            )
                    v8 = apool.tile([128, 512], f8, tag="kv8", name="v8")
                    nc.vector.tensor_scalar_mul(v8, pv, IWS)
                    nc.sync.dma_start(vin[tsub * 128 : (tsub + 1) * 128, :], v8)
                nc.gpsimd.collective_compute(
                    "AllGather",
                    OP.bypass,
                    replica_groups=RG,
                    ins=[kv_in.opt()],
                    outs=[kv_g.opt()],
                )
                kv_ga.append(kv_g)

            # ---------- Q projection (overlaps AllGathers) ----------
            wq_sb = wpool.tile([128, KSUB, C], f8, tag="wmat", name="wq_sb")
            nc.sync.dma_start(wq_sb, wq_d[layer])
            qT = apool.tile([128, NH, TOK], f8, tag="qT", name="qT", bufs=1)
            for hh in range(NH):
                pq = ps_s.tile([128, TOK], dt.float32, tag="s", name="pq")
                if fp8:
                    for gp in range(4):
                        nc.tensor.matmul(
                            pq,
                            wq_sb[:, 2 * gp : 2 * gp + 2, hh * HD : (hh + 1) * HD],
                            aT[:, 2 * gp : 2 * gp + 2, :],
                            start=(gp == 0), stop=(gp == 3), perf_mode=DR,
                        )
                else:
                    for ct in range(KSUB):
                        nc.tensor.matmul(
                            pq, wq_sb[:, ct, hh * HD : (hh + 1) * HD], aT[:, ct, :],
                            start=(ct == 0), stop=(ct == KSUB - 1),
                        )
                nc.vector.tensor_scalar_mul(qT[:, hh, :], pq, IWS)

            # ---------- attention ----------
            yT = apool.tile([128, NH, TOK], f8, tag="yT", name="yT", bufs=1)
            for grp in range(2):
                kv_g = kv_ga[grp]
                kg_all = apool.tile([128, 2, 4, TOK], f8, tag="kg", name="kg_all")
                vg_all = apool.tile([128, 8, 512], f8, tag="vg", name="vg_all")
                for r in range(2):
                    kga = kv_g[r, 0].rearrange("(hl hd t) -> hd hl t", hd=HD, t=TOK)
                    vga = kv_g[r, 1].rearrange("(ts t c) -> t ts c", t=128, c=4 * HD)
                    nc.sync.dma_start(kg_all[:, r, :, :], kga)
                    nc.sync.dma_start(vg_all[:, r * 4 : (r + 1) * 4, :], vga)
                def head_tail(hh, den_ps, py):
                    den16 = apool.tile([1, TOK], dt.bfloat16, tag="den16",
                                       name="den16")
                    nc.vector.tensor_copy(den16, den_ps)
                    den_b = ps_v.tile([128, TOK], dt.float32, tag="v", name="den_b")
                    nc.tensor.matmul(den_b, onesrow, den16, start=True, stop=True)
                    recB = apool.tile([128, TOK], dt.float32, tag="recB",
                                      name="recB")
                    nc.vector.reciprocal_approx_fast(recB, den_b)
                    nc.vector.tensor_mul(yT[:, hh, :], py, recB)

                pending = None
                for hl in range(4):
                    hh = grp * 4 + hl

                    e_sb = apool.tile([128, 8, TOK], f8, tag="e_sb", name="e_sb", bufs=3)
                    den_ps = ps_v.tile([1, TOK], dt.float32, tag="v", name="den_ps")
                    py = ps_acc.tile([128, TOK], dt.float32, tag="acc", name="py")

                    def den_av(p):
                        # den/AV accumulation, deferred 3 positions behind the
                        # scores so the PE never blocks on exp (FIFO queues).
                        # AV: odd p pairs with p-1 in DoubleRow over the
                        # narrower suffix; the 128-col head of the even
                        # position is done in normal mode.
                        S = SLAB[p]
                        nc.tensor.matmul(
                            den_ps[:, S:TOK], ones8, e_sb[:, p, S:TOK],
                            start=(p == 0), stop=(p == 7),
                        )
                        nc.tensor.matmul(
                            py[:, S:TOK],
                            vg_all[:, p, hl * HD : (hl + 1) * HD],
                            e_sb[:, p, S:TOK],
                            start=(p == 0), stop=(p == 7),
                        )

                    for p in range(8):
                        S = SLAB[p]
                        s_ps = ps_s.tile([128, TOK], dt.float32, tag="s", name="s_ps")
                        nc.tensor.matmul(
                            s_ps[:, S:TOK],
                            kg_all[:, p // 4, hl, (p % 4) * 128 : (p % 4 + 1) * 128],
                            qT[:, hh, S:TOK],
                            start=True, stop=True,
                        )
                        if p == 4:
                            nc.vector.tensor_add(
                                s_ps[0:64, 0:64], s_ps[0:64, 0:64], pb_sb[:, hh, :]
                            )
                        nc.scalar.activation(
                            e_sb[:, p, S:TOK], s_ps[:, S:TOK], AF.Exp,
                            bias=cb_sb, scale=1.0 / SQHD,
                        )
                        nc.vector.tensor_mul(
                            e_sb[:, p, S : S + 128],
                            e_sb[:, p, S : S + 128],
                            dm_sb[:, p, :],
                        )
                        if p >= 3:
                            den_av(p - 3)
                    for p in range(5, 8):
                        den_av(p)
                    if pending is not None:
                        head_tail(*pending)
                    pending = (hh, den_ps, py)
                head_tail(*pending)

            # ---------- proj + residual ----------
            wp_sb = wpool.tile([128, KSUB, C], f8, tag="wmat", name="wp_sb")
            nc.sync.dma_start(wp_sb, wp_d[layer])
            for co in range(KSUB):
                pp = ps_s.tile([128, TOK], dt.float32, tag="s", name="pp")
                if fp8:
                    for gp in range(4):
                        nc.tensor.matmul(
                            pp,
                            wp_sb[:, 2 * gp : 2 * gp + 2, co * 128 : (co + 1) * 128],
                            yT[:, 2 * gp : 2 * gp + 2, :],
                            start=(gp == 0), stop=(gp == 3), perf_mode=DR,
                        )
                else:
                    for ct in range(KSUB):
                        nc.tensor.matmul(
                            pp, wp_sb[:, ct, co * 128 : (co + 1) * 128], yT[:, ct, :],
                            start=(ct == 0), stop=(ct == KSUB - 1),
                        )
                resid_add(co, pp, IWS)

            # ---------- LN2 ----------
            a2T = apool.tile([128, KSUB, TOK], dt.bfloat16, tag="aT", name="a2T")
            layernorm(a2T)

            # ---------- MLP (bf16) ----------
            g_sb = apool.tile([128, HSUB, TOK], dt.bfloat16, tag="g_sb", name="g_sb",
                              bufs=1)
            for hblk in range(8):
                w1_sb = w1pool.tile([128, KSUB, 512], dt.bfloat16, tag="w1b",
                                    name="w1_sb")
                nc.sync.dma_start(w1_sb, w1_d[layer, hblk])
                for hc in range(4):
                    pu = ps_s.tile([128, TOK], dt.float32, tag="s", name="pu")
                    for ct in range(KSUB):
                        nc.tensor.matmul(
                            pu,
                            w1_sb[:, ct, hc * 128 : (hc + 1) * 128],
                            a2T[:, ct, :],
                            start=(ct == 0), stop=(ct == KSUB - 1),
                        )
                    nc.scalar.activation(g_sb[:, hblk * 4 + hc, :], pu, AF.Gelu)

            for grp2 in range(4):
                pd = [
                    ps_acc.tile([128, TOK], dt.float32, tag="acc", name=f"pd{i}")
                    for i in range(2)
                ]
                for ksub in range(HSUB):
                    w2_sb = w2pool.tile([128, 256], dt.bfloat16, tag="w2t",
                                        name="w2_sb")
                    nc.sync.dma_start(w2_sb, w2_d[layer, grp2, ksub])
                    for i in range(2):
                        nc.tensor.matmul(
                            pd[i],
                            w2_sb[:, i * 128 : (i + 1) * 128],
                            g_sb[:, ksub, :],
                            start=(ksub == 0), stop=(ksub == HSUB - 1),
                        )
                for i in range(2):
                    resid_add(grp2 * 2 + i, pd[i], 1.0)

        # ---------- final LN + head (bf16) ----------
        hfT = apool.tile([128, KSUB, TOK], dt.bfloat16, tag="aT", name="hfT")
        layernorm(hfT)
        hfT_r = hfT.rearrange("p k (b e) -> p k e b", e=TD)  # b: 8 blocks of 64
        NB = TOK // TD  # 8 tokens per class
        for eg in range(TD // 4):  # 4 e-classes concurrently via PE col groups
            hw4 = [
                w1pool.tile([128, KSUB, 128], dt.bfloat16, tag=f"hw{j}",
                            name=f"hw4_{j}", bufs=4)
                for j in range(4)
            ]
            for j in range(4):
                nc.sync.dma_start(hw4[j], hwT_d[4 * eg + j])
            po4 = ps_s.tile([128, V1], dt.float32, tag="s", name="po4")
            for ct in range(KSUB):
                for j in range(4):
                    nc.tensor.matmul(
                        po4[32 * j : 32 * j + NB, :],
                        hfT_r[:, ct, 4 * eg + j, :],
                        hw4[j][:, ct, 0:V1],
                        start=(ct == 0), stop=(ct == KSUB - 1),
                        tile_position=(0, 32 * j),
                    )
            o_sb = apool.tile([128, V1], dt.float32, tag="o_sb", name="o_sb")
            nc.vector.tensor_copy(o_sb, po4)
            for j in range(4):
                nc.sync.dma_start(out_d[4 * eg + j], o_sb[32 * j : 32 * j + NB, :])

        for p in (ps_v, ps_s, ps_acc, dpool, apool, w2pool, w1pool, wpool, hpool,
                  consts):
            p.release()

    nc.compile()
    return nc


def _tok_idx(half):
    tiles = TILES_A if half == 0 else TILES_B
    return np.concatenate([np.arange(t * 128, (t + 1) * 128) for t in tiles])


def _host_inputs(x, attn_bias, pos_emb, Wq, Wk, Wv, Wp, w1, w2, head_w, fp8=True):
    f8c = (lambda a: np.clip(a, -240, 240).astype(FP8)) if fp8 else (
        lambda a: a.astype(BF16))

    def packw(W):
        # (L, C, N) -> (L, 128, KSUB, N): partition-major contiguous
        W = np.asarray(W, np.float32)
        n = W.shape[2]
        return np.ascontiguousarray(
            W.reshape(L, KSUB, 128, n).transpose(0, 2, 1, 3))

    sc = WS if fp8 else 1.0
    wq = f8c(packw(Wq) * sc)
    wk = f8c(packw(Wk) * sc)
    wv = f8c(packw(Wv) * sc)
    wp = f8c(packw(Wp) * sc)
    w1b = np.ascontiguousarray(
        np.asarray(w1, np.float32).reshape(L, KSUB, 128, 8, 512)
        .transpose(0, 3, 2, 1, 4)).astype(BF16)
    w2b = np.ascontiguousarray(
        np.asarray(w2, np.float32).reshape(L, HSUB, 128, 4, 256)
        .transpose(0, 3, 1, 2, 4)).astype(BF16)
    hwT = np.zeros((TD, C, 128), np.float32)
    hwT[:, :, :V1] = np.asarray(head_w, np.float32).transpose(0, 2, 1)
    hwT = np.ascontiguousarray(
        hwT.reshape(TD, KSUB, 128, 128).transpose(0, 2, 1, 3)).astype(BF16)

    # graph bias, transposed (kv, head, q), padded 60->64, prescaled by
    # sqrt(HD) (the kernel applies exp(s/sqrt(HD) + colbias))
    bias = np.repeat(np.repeat(np.asarray(attn_bias, np.float32), F_DIM, 1), F_DIM, 2)
    pbT = np.zeros((64, NH, 64), np.float32)
    pbT[:D_BIAS, :, :D_BIAS] = bias.transpose(2, 0, 1) * SQHD  # [j, h, i]
    pbT_zero = np.zeros_like(pbT)

    # column-disable bias: kv rows with global_j % 64 == 63 get -50
    cb = np.zeros((128, 1), np.float32)
    cb[63, 0] = -50.0
    cb[127, 0] = -50.0

    # per-position first-slab masks [128(k), 8(pos), 128(q)]
    tri = np.tril(np.ones((128, 128), np.float32)).T  # tri[k, q] = k <= q
    onesm = np.ones((128, 128), np.float32)
    zerom = np.zeros((128, 128), np.float32)
    dm_A = np.stack([tri, tri, tri, tri, onesm, zerom, zerom, onesm], 1)
    dm_B = np.stack([zerom, onesm, onesm, zerom, tri, tri, tri, tri], 1)
    dm_A = np.ascontiguousarray(dm_A).astype(BF16)
    dm_B = np.ascontiguousarray(dm_B).astype(BF16)

    h0 = np.asarray(x, np.float32) + np.asarray(pos_emb, np.float32)  # (B, T, C)

    in_maps = []
    for core in range(NCORES):
        b, half = core // 2, core % 2
        idx = _tok_idx(half)
        h0T = np.ascontiguousarray(
            h0[b, idx].T.reshape(KSUB, 128, TOK).transpose(1, 0, 2)
        ).astype(BF16)  # (128, KSUB, TOK)
        in_maps.append(
            {
                "h0T": h0T,
                "dm": dm_A if half == 0 else dm_B,
                "pbT": pbT_zero if half == 0 else pbT,
                "cb": cb,
                "wq": wq, "wk": wk, "wv": wv, "wp": wp,
                "w1": w1b, "w2": w2b, "hwT": hwT,
            }
        )
    return in_maps


def kernel(**inputs):
    from concourse.bass_utils import run_bass_kernel_spmd

    fp8 = bool(int(os.environ.get("KERNEL_FP8", "1")))
    in_maps = _host_inputs(
        inputs["x"], inputs["attn_bias"], inputs["pos_emb"],
        inputs["Wq"], inputs["Wk"], inputs["Wv"], inputs["Wp"],
        inputs["w1"], inputs["w2"], inputs["head_w"], fp8=fp8,
    )
    if "nc" not in _CACHED:
        _CACHED["nc"] = _build_program(fp8=fp8)
    res = run_bass_kernel_spmd(
        _CACHED["nc"], in_maps, core_ids=list(range(NCORES)),
        trace=bool(int(os.environ.get("KERNEL_TRACE", "0"))),
    )
    out = np.zeros((B, T, V1), np.float32)
    for core in range(NCORES):
        b, half = core // 2, core % 2
        lg = res.results[core]["logits"]  # (TD, 8, V1): token = b*64 + e
        lg = lg.transpose(1, 0, 2).reshape(TOK, V1)
        out[b, _tok_idx(half)] = lg
    _CACHED["last_result"] = res
    return out
